# revision 28
# baseline (speedup 1.0000x reference)
"""Multi-head attention (B=2, N=2048, D=1024, H=16) on 8 trn2 cores.

Sharding: tensor-parallel over heads — each core computes 2 heads' QKV
projections + attention + its partial W_o product; the host sums the 8
partials (the all-reduce of the W_o row-sharding, done at unshard time).

Per-core DRAM layouts (feature-major / transposed):
  xT   [1024 d_in, 4096 tok]          bf16
  wq/wk/wv [1024 d_in, 128 f]         bf16 (f = 2 heads x 64, head-major)
  wo   [130, 1024]                    bf16 (two 65-row blocks per head:
                                       row 0 zero, rows 1-64 = W_o slice)
  out  [1024 d_out, 4096 tok]         fp32 (partial; host sums over cores)

Attention computes scores transposed ([keys, queries] in PSUM) so the
softmax denominator is a matmul contraction: V carries a leading ones
column (lhsT [128 k, 65] = [ones | V_h]), so the attn@V accumulator's
row 0 is the denominator and rows 1-64 the weighted values. The
normalization multiplies rows 0-64 by a rank-1 broadcast of 1/denom
(row 0 becomes 1, absorbed by the zero row in wo).
"""
import sys

sys.path.insert(0, "/opt/trn_rl_repo")

import numpy as np
import ml_dtypes

B, N, D, H = 2, 2048, 1024, 16
HD = D // H          # 64
N_CORES = 8
HPC = H // N_CORES   # heads per core = 2
F = HPC * HD         # per-core features = 128
T = B * N            # tokens = 4096
TCH = 512            # token chunk (QKV phase, q-window)
KT = 128             # key tile
NKT = N // KT        # 16 key tiles per batch
SCALE = 1.0 / np.sqrt(HD)

_BUILT = None


def _build():
    import concourse.tile as tile
    from concourse import bacc, mybir

    bf16 = mybir.dt.bfloat16
    f32 = mybir.dt.float32

    nc = bacc.Bacc("TRN2", target_bir_lowering=False, debug=False,
                   num_devices=N_CORES)
    xT_d = nc.dram_tensor("xT", [D, T], bf16, kind="ExternalInput").ap()
    wq_d = nc.dram_tensor("wq", [D, F], bf16, kind="ExternalInput").ap()
    wk_d = nc.dram_tensor("wk", [D, F], bf16, kind="ExternalInput").ap()
    wv_d = nc.dram_tensor("wv", [D, F], bf16, kind="ExternalInput").ap()
    wo_d = nc.dram_tensor("wo", [F, D], bf16, kind="ExternalInput").ap()
    out_d = nc.dram_tensor("out", [D, T], f32, kind="ExternalOutput").ap()

    DCH = D // 128  # 8 contraction chunks for the projections

    with tile.TileContext(nc) as tc:
        with (
            tc.tile_pool(name="wpool", bufs=1) as wpool,
            tc.tile_pool(name="xin", bufs=32) as xin,
            tc.tile_pool(name="qkv", bufs=1) as qkv,
            tc.tile_pool(name="expp", bufs=3) as expp,
            tc.tile_pool(name="small", bufs=4) as small,
            tc.tile_pool(name="ps", bufs=2, space="PSUM") as ps,
        ):
            # --- weights to SBUF ---
            wq_sb = wpool.tile([128, D], bf16, tag="wq")   # [p, dchunk*128+f]
            wk_sb = wpool.tile([128, D], bf16, tag="wk")
            wv_sb = wpool.tile([128, D], bf16, tag="wv")
            wo_sb = wpool.tile([F, D], bf16, tag="wo")
            for w_sb, w_d in ((wq_sb, wq_d), (wk_sb, wk_d), (wv_sb, wv_d)):
                nc.gpsimd.dma_start(
                    w_sb.rearrange("p (c f) -> p c f", f=F),
                    w_d.rearrange("(c p) f -> p c f", p=128))
            nc.gpsimd.dma_start(wo_sb[:], wo_d[:])
            ones_sb = wpool.tile([1, HD + 1], f32, tag="ones")
            nc.vector.memset(ones_sb[:], 1.0)

            # --- per-batch persistent tensors ---
            q_sb = [qkv.tile([128, N], bf16, tag=f"q{b}", name=f"q{b}")
                    for b in range(B)]
            k_sb = [qkv.tile([128, N], bf16, tag=f"k{b}", name=f"k{b}")
                    for b in range(B)]
            # v: per batch [128 tok, 16 ktiles x 2 heads x (ones|64 vals)]
            VW = NKT * 2 * (HD + 1)  # 2080
            v_sb = [qkv.tile([128, VW], bf16, tag=f"v{b}", name=f"v{b}")
                    for b in range(B)]
            # per-head normalized output (row 0 garbage), then DMA-shifted
            # into the combined concat [128 f, N] for a single K=128 W_o
            ch_sb = [[qkv.tile([HD + 1, N], bf16, tag=f"h{b}{h}",
                               name=f"h{b}{h}")
                      for h in range(HPC)] for b in range(B)]
            cc_sb = [qkv.tile([F, N], bf16, tag=f"c{b}", name=f"c{b}")
                     for b in range(B)]

            for b in range(B):
                nc.vector.memset(
                    v_sb[b].rearrange("p (g c) -> p g c", c=HD + 1)[:, :, 0:1],
                    1.0)

            # --- QKV projections for one batch. Loop d-chunk OUTER so the
            # stationary weight chunk is loaded once per 4 token-chunk
            # matmuls (LDWEIGHTS exposure was ~40% of PE time). V is
            # computed transposed the same way, then DMA-transposed into
            # token-major layout on the (idle) GpSimd queues. ---
            def emit_qkv(b):
                xt = [[xin.tile([128, TCH], bf16, tag="x",
                                name=f"x{b}_{wc}_{c}")
                       for c in range(DCH)] for wc in range(N // TCH)]
                for wc in range(N // TCH):
                    toff = b * N + wc * TCH
                    for c in range(DCH):
                        nc.sync.dma_start(
                            xt[wc][c][:], xT_d[c * 128:(c + 1) * 128,
                                               toff:toff + TCH])
                for i, (w_sb, dst) in enumerate(
                        ((wq_sb, q_sb[b]), (wk_sb, k_sb[b]))):
                    accs = [ps.tile([128, 1024], f32, tag="big",
                                    name=f"qk{b}_{i}_{s}")
                            for s in range(2)]
                    for c in range(DCH):
                        for wc in range(N // TCH):
                            a = accs[wc // 2]
                            nc.tensor.matmul(
                                a[:, (wc % 2) * TCH:(wc % 2 + 1) * TCH],
                                w_sb[:, c * 128:(c + 1) * 128],
                                xt[wc][c][:],
                                start=(c == 0), stop=(c == DCH - 1))
                    for s in range(2):
                        nc.vector.tensor_copy(
                            dst[:, s * 1024:(s + 1) * 1024], accs[s][:])
                # V: [tok, f] layout, per 128-token tile
                for wc in range(N // TCH):
                    for tt4 in range(TCH // 128):
                        tt = wc * 4 + tt4
                        acc = ps.tile([128, 1024], f32, tag="big",
                                      name=f"vp{b}_{tt}")
                        for c in range(DCH):
                            nc.tensor.matmul(
                                acc[:, 0:128],
                                xt[wc][c][:, tt4 * 128:(tt4 + 1) * 128],
                                wv_sb[:, c * 128:(c + 1) * 128],
                                start=(c == 0), stop=(c == DCH - 1))
                        for h in range(HPC):
                            g = tt * 2 * (HD + 1) + h * (HD + 1)
                            nc.vector.tensor_copy(
                                v_sb[b][:, g + 1:g + HD + 1],
                                acc[:, h * HD:(h + 1) * HD])

            # --- attention, all 8 (batch, window) pairs, with the
            # normalize/W_o tail of window w-1 pipelined into window w's
            # k-loop so the PE never stalls on the DVE/DMA chain ---
            def emit_normalize(b, wc):
                qoff = wc * TCH
                av = av_tiles[(b, wc)]
                for h in range(HPC):
                    rec = small.tile([1, TCH], f32, tag="rec",
                                     name=f"rec{b}_{wc}_{h}")
                    nc.vector.reciprocal_approx_fast(rec[:], av[h][0:1, :])
                    bc = ps.tile([HD + 1, TCH], f32, tag="wo",
                                 name=f"bc{b}_{wc}_{h}")
                    nc.tensor.matmul(bc[:], ones_sb[:], rec[:],
                                     start=True, stop=True)
                    bcs = small.tile([HD + 1, TCH], f32, tag="bcs",
                                     name=f"bcs{b}_{wc}_{h}")
                    nc.vector.tensor_copy(bcs[:], bc[:])
                    nc.vector.tensor_mul(
                        ch_sb[b][h][:, qoff:qoff + TCH], av[h][:], bcs[:])
                    # partition-shift rows 1-64 -> combined concat
                    nc.sync.dma_start(
                        cc_sb[b][h * HD:(h + 1) * HD, qoff:qoff + TCH],
                        ch_sb[b][h][1:HD + 1, qoff:qoff + TCH])

            def emit_wo(b, wc):
                qoff = wc * TCH
                for j in range(DCH):
                    acc = ps.tile([128, TCH], f32, tag="wo",
                                  name=f"wops{b}_{wc}_{j}")
                    nc.tensor.matmul(
                        acc[:],
                        wo_sb[:, j * 128:(j + 1) * 128],
                        cc_sb[b][:, qoff:qoff + TCH],
                        start=True, stop=True)
                    od = small.tile([128, TCH], f32, tag="od",
                                    name=f"od{b}_{wc}_{j}")
                    nc.vector.tensor_copy(od[:], acc[:])
                    nc.sync.dma_start(
                        out_d[j * 128:(j + 1) * 128,
                              b * N + qoff:b * N + qoff + TCH],
                        od[:])

            av_tiles = {}
            prev = None
            for b in range(B):
                emit_qkv(b)
                for wc in range(N // TCH):
                    qoff = wc * TCH
                    av = [ps.tile([HD + 1, TCH], f32, tag="av",
                                  name=f"av{b}_{wc}_{h}")
                          for h in range(HPC)]
                    av_tiles[(b, wc)] = av
                    for kt in range(NKT):
                        koff = kt * KT
                        sc = ps.tile([128, 1024], f32, tag="big",
                                     name=f"sc{b}_{wc}_{kt}")
                        for h in range(HPC):
                            nc.tensor.matmul(
                                sc[:, h * TCH:(h + 1) * TCH],
                                k_sb[b][h * HD:(h + 1) * HD, koff:koff + KT],
                                q_sb[b][h * HD:(h + 1) * HD, qoff:qoff + TCH],
                                start=True, stop=True)
                        ex = expp.tile([128, 1024], bf16, tag="e",
                                       name=f"ex{b}_{wc}_{kt}")
                        nc.scalar.activation(
                            ex[:], sc[:], mybir.ActivationFunctionType.Exp,
                            scale=float(SCALE))
                        for h in range(HPC):
                            g = kt * 2 * (HD + 1) + h * (HD + 1)
                            nc.tensor.matmul(
                                av[h][:],
                                v_sb[b][:, g:g + HD + 1],
                                ex[:, h * TCH:(h + 1) * TCH],
                                start=(kt == 0), stop=(kt == NKT - 1))
                        if kt == 5 and prev is not None:
                            emit_normalize(*prev)
                        if kt == 14 and prev is not None:
                            emit_wo(*prev)
                            prev = None
                    prev = (b, wc)
            emit_normalize(*prev)
            emit_wo(*prev)
    nc.compile()
    return nc


def _get_built():
    global _BUILT
    if _BUILT is None:
        _BUILT = _build()
    return _BUILT


_RUNNER = None


def _get_runner():
    """Build the sharded jitted executable once (jax.jit caches on function
    identity, so run_bass_kernel_spmd would re-trace every call)."""
    global _RUNNER
    if _RUNNER is not None:
        return _RUNNER
    import jax
    import numpy as _np
    from jax.sharding import Mesh, PartitionSpec
    from jax.experimental.shard_map import shard_map
    from concourse import mybir
    from concourse.bass2jax import (
        install_neuronx_cc_hook, _bass_exec_p, partition_id_tensor)

    nc = _get_built()
    install_neuronx_cc_hook()

    partition_name = (nc.partition_id_tensor.name
                      if nc.partition_id_tensor else None)
    in_names, out_names, out_avals, zero_shapes = [], [], [], []
    for alloc in nc.m.functions[0].allocations:
        if not isinstance(alloc, mybir.MemoryLocationSet):
            continue
        name = alloc.memorylocations[0].name
        if alloc.kind == "ExternalInput":
            if name != partition_name:
                in_names.append(name)
        elif alloc.kind == "ExternalOutput":
            np_dt = mybir.dt.np(alloc.dtype)
            out_avals.append(
                jax.core.ShapedArray(tuple(alloc.tensor_shape), np_dt))
            out_names.append(name)
            zero_shapes.append((tuple(alloc.tensor_shape), np_dt))
    n_params = len(in_names)
    all_names = in_names + out_names
    if partition_name is not None:
        all_names = all_names + [partition_name]
    # xT is identical on every core — pass it replicated
    repl = {"xT"}
    import jax.numpy as jnp
    from jax.sharding import NamedSharding

    def _body(*args):
        operands = list(args)
        if partition_name is not None:
            operands.append(partition_id_tensor())
        outs = _bass_exec_p.bind(
            *operands,
            out_avals=tuple(out_avals),
            in_names=tuple(all_names),
            out_names=tuple(out_names),
            lowering_input_output_aliases=(),
            sim_require_finite=True,
            sim_require_nnan=True,
            nc=nc,
        )
        return tuple(outs)

    devices = jax.devices()[:N_CORES]
    mesh = Mesh(_np.asarray(devices), ("core",))
    n_outs = len(out_names)
    in_specs = tuple(
        PartitionSpec() if nm in repl else PartitionSpec("core")
        for nm in in_names) + (PartitionSpec("core"),) * n_outs
    sharded = jax.jit(
        shard_map(_body, mesh=mesh, in_specs=in_specs,
                  out_specs=(PartitionSpec("core"),) * n_outs,
                  check_rep=False),
        donate_argnums=tuple(range(n_params, n_params + n_outs)),
        keep_unused=True,
    )
    shard0 = NamedSharding(mesh, PartitionSpec("core"))
    zeros_jit = jax.jit(
        lambda: tuple(jnp.zeros((N_CORES * s[0], *s[1:]), dt)
                      for s, dt in zero_shapes),
        out_shardings=tuple(shard0 for _ in zero_shapes))
    # sum the 8 partial W_o products on-device (all-reduce), return one copy
    reduce_jit = jax.jit(
        lambda o: jnp.sum(o.reshape(N_CORES, *zero_shapes[0][0]), axis=0),
        out_shardings=NamedSharding(mesh, PartitionSpec()))

    def run(in_maps):
        args = []
        for nm in in_names:
            if nm in repl:
                args.append(in_maps[0][nm])
            else:
                args.append(_np.concatenate(
                    [in_maps[c][nm] for c in range(N_CORES)], axis=0))
        outs = sharded(*args, *zeros_jit())
        return _np.asarray(reduce_jit(outs[0]))

    _RUNNER = run
    return run


def _head_rows(h):
    # feature d of head h sits at column i*H + h of the QKV projection
    # output (einops 'b n (d h)' with head as the inner factor)
    return np.arange(HD) * H + h


def shard_inputs(x, W_q, W_k, W_v, W_o):
    bf = ml_dtypes.bfloat16
    xT = np.ascontiguousarray(
        np.asarray(x, dtype=np.float32).reshape(T, D).T).astype(bf)
    W_q, W_k, W_v, W_o = (np.asarray(w, dtype=np.float32)
                          for w in (W_q, W_k, W_v, W_o))
    in_maps = []
    for c in range(N_CORES):
        rows = np.concatenate([_head_rows(2 * c), _head_rows(2 * c + 1)])
        # concat ('b h n d -> b n (h d)') puts head h's features at
        # columns [h*64, (h+1)*64)
        cols = np.arange(2 * c * HD, (2 * c + 2) * HD)
        in_maps.append({
            "xT": xT,
            "wq": np.ascontiguousarray(W_q[rows, :].T).astype(bf),
            "wk": np.ascontiguousarray(W_k[rows, :].T).astype(bf),
            "wv": np.ascontiguousarray(W_v[rows, :].T).astype(bf),
            "wo": np.ascontiguousarray(W_o[:, cols].T).astype(bf),
        })
    return in_maps


def kernel(x, W_q, W_k, W_v, W_o):
    run = _get_runner()
    total = run(shard_inputs(x, W_q, W_k, W_v, W_o))
    return np.ascontiguousarray(total.T).reshape(B, N, D)


# revision 29
# speedup vs baseline: 1.0048x; 1.0048x over previous
"""Multi-head attention (B=2, N=2048, D=1024, H=16) on 8 trn2 cores.

Sharding: tensor-parallel over heads — each core computes 2 heads' QKV
projections + attention + its partial W_o product; the host sums the 8
partials (the all-reduce of the W_o row-sharding, done at unshard time).

Per-core DRAM layouts (feature-major / transposed):
  xT   [1024 d_in, 4096 tok]          bf16
  wq/wk/wv [1024 d_in, 128 f]         bf16 (f = 2 heads x 64, head-major)
  wo   [130, 1024]                    bf16 (two 65-row blocks per head:
                                       row 0 zero, rows 1-64 = W_o slice)
  out  [1024 d_out, 4096 tok]         fp32 (partial; host sums over cores)

Attention computes scores transposed ([keys, queries] in PSUM) so the
softmax denominator is a matmul contraction: V carries a leading ones
column (lhsT [128 k, 65] = [ones | V_h]), so the attn@V accumulator's
row 0 is the denominator and rows 1-64 the weighted values. The
normalization multiplies rows 0-64 by a rank-1 broadcast of 1/denom
(row 0 becomes 1, absorbed by the zero row in wo).
"""
import sys

sys.path.insert(0, "/opt/trn_rl_repo")

import numpy as np
import ml_dtypes

B, N, D, H = 2, 2048, 1024, 16
HD = D // H          # 64
N_CORES = 8
HPC = H // N_CORES   # heads per core = 2
F = HPC * HD         # per-core features = 128
T = B * N            # tokens = 4096
TCH = 512            # token chunk (QKV phase, q-window)
KT = 128             # key tile
NKT = N // KT        # 16 key tiles per batch
SCALE = 1.0 / np.sqrt(HD)

_BUILT = None


def _build():
    import concourse.tile as tile
    from concourse import bacc, mybir

    bf16 = mybir.dt.bfloat16
    f32 = mybir.dt.float32

    nc = bacc.Bacc("TRN2", target_bir_lowering=False, debug=False,
                   num_devices=N_CORES)
    xT_d = nc.dram_tensor("xT", [D, T], bf16, kind="ExternalInput").ap()
    wq_d = nc.dram_tensor("wq", [D, F], bf16, kind="ExternalInput").ap()
    wk_d = nc.dram_tensor("wk", [D, F], bf16, kind="ExternalInput").ap()
    wv_d = nc.dram_tensor("wv", [D, F], bf16, kind="ExternalInput").ap()
    wo_d = nc.dram_tensor("wo", [F, D], bf16, kind="ExternalInput").ap()
    out_d = nc.dram_tensor("out", [D, T], f32, kind="ExternalOutput").ap()

    DCH = D // 128  # 8 contraction chunks for the projections

    with tile.TileContext(nc) as tc:
        with (
            tc.tile_pool(name="wpool", bufs=1) as wpool,
            tc.tile_pool(name="xin", bufs=16) as xin,
            tc.tile_pool(name="qkv", bufs=1) as qkv,
            tc.tile_pool(name="expp", bufs=3) as expp,
            tc.tile_pool(name="small", bufs=4) as small,
            tc.tile_pool(name="ps", bufs=2, space="PSUM") as ps,
        ):
            # --- weights to SBUF ---
            wq_sb = wpool.tile([128, D], bf16, tag="wq")   # [p, dchunk*128+f]
            wk_sb = wpool.tile([128, D], bf16, tag="wk")
            wv_sb = wpool.tile([128, D], bf16, tag="wv")
            wo_sb = wpool.tile([F, D], bf16, tag="wo")
            for w_sb, w_d in ((wq_sb, wq_d), (wk_sb, wk_d), (wv_sb, wv_d)):
                nc.gpsimd.dma_start(
                    w_sb.rearrange("p (c f) -> p c f", f=F),
                    w_d.rearrange("(c p) f -> p c f", p=128))
            nc.gpsimd.dma_start(wo_sb[:], wo_d[:])
            ones_sb = wpool.tile([1, HD + 1], f32, tag="ones")
            nc.vector.memset(ones_sb[:], 1.0)

            # --- per-batch persistent tensors ---
            q_sb = [qkv.tile([128, N], bf16, tag=f"q{b}", name=f"q{b}")
                    for b in range(B)]
            k_sb = [qkv.tile([128, N], bf16, tag=f"k{b}", name=f"k{b}")
                    for b in range(B)]
            # v: per batch [128 tok, 16 ktiles x 2 heads x (ones|64 vals)]
            VW = NKT * 2 * (HD + 1)  # 2080
            v_sb = [qkv.tile([128, VW], bf16, tag=f"v{b}", name=f"v{b}")
                    for b in range(B)]
            # per-head normalized output (row 0 garbage), then DMA-shifted
            # into the combined concat [128 f, N] for a single K=128 W_o
            ch_sb = [[qkv.tile([HD + 1, N], bf16, tag=f"h{b}{h}",
                               name=f"h{b}{h}")
                      for h in range(HPC)] for b in range(B)]
            cc_sb = [qkv.tile([F, N], bf16, tag=f"c{b}", name=f"c{b}")
                     for b in range(B)]

            for b in range(B):
                nc.vector.memset(
                    v_sb[b].rearrange("p (g c) -> p g c", c=HD + 1)[:, :, 0:1],
                    1.0)

            # --- QKV projections for one batch. Loop d-chunk OUTER so the
            # stationary weight chunk is loaded once per 4 token-chunk
            # matmuls (LDWEIGHTS exposure was ~40% of PE time). V is
            # computed transposed the same way, then DMA-transposed into
            # token-major layout on the (idle) GpSimd queues. ---
            def emit_qkv(b):
                xb = [xin.tile([128, N], bf16, tag="x", name=f"x{b}_{c}")
                      for c in range(DCH)]
                for c in range(DCH):
                    nc.sync.dma_start(
                        xb[c][:], xT_d[c * 128:(c + 1) * 128,
                                       b * N:(b + 1) * N])
                xt = [[xb[c][:, wc * TCH:(wc + 1) * TCH]
                       for c in range(DCH)] for wc in range(N // TCH)]
                for i, (w_sb, dst) in enumerate(
                        ((wq_sb, q_sb[b]), (wk_sb, k_sb[b]))):
                    accs = [ps.tile([128, 1024], f32, tag="big",
                                    name=f"qk{b}_{i}_{s}")
                            for s in range(2)]
                    for c in range(DCH):
                        for wc in range(N // TCH):
                            a = accs[wc // 2]
                            nc.tensor.matmul(
                                a[:, (wc % 2) * TCH:(wc % 2 + 1) * TCH],
                                w_sb[:, c * 128:(c + 1) * 128],
                                xt[wc][c][:],
                                start=(c == 0), stop=(c == DCH - 1))
                    for s in range(2):
                        nc.vector.tensor_copy(
                            dst[:, s * 1024:(s + 1) * 1024], accs[s][:])
                # V: [tok, f] layout, per 128-token tile
                for wc in range(N // TCH):
                    for tt4 in range(TCH // 128):
                        tt = wc * 4 + tt4
                        acc = ps.tile([128, 1024], f32, tag="big",
                                      name=f"vp{b}_{tt}")
                        for c in range(DCH):
                            nc.tensor.matmul(
                                acc[:, 0:128],
                                xt[wc][c][:, tt4 * 128:(tt4 + 1) * 128],
                                wv_sb[:, c * 128:(c + 1) * 128],
                                start=(c == 0), stop=(c == DCH - 1))
                        for h in range(HPC):
                            g = tt * 2 * (HD + 1) + h * (HD + 1)
                            nc.vector.tensor_copy(
                                v_sb[b][:, g + 1:g + HD + 1],
                                acc[:, h * HD:(h + 1) * HD])

            # --- attention, all 8 (batch, window) pairs, with the
            # normalize/W_o tail of window w-1 pipelined into window w's
            # k-loop so the PE never stalls on the DVE/DMA chain ---
            def emit_normalize(b, wc):
                qoff = wc * TCH
                av = av_tiles[(b, wc)]
                for h in range(HPC):
                    rec = small.tile([1, TCH], f32, tag="rec",
                                     name=f"rec{b}_{wc}_{h}")
                    nc.vector.reciprocal_approx_fast(rec[:], av[h][0:1, :])
                    bc = ps.tile([HD + 1, TCH], f32, tag="wo",
                                 name=f"bc{b}_{wc}_{h}")
                    nc.tensor.matmul(bc[:], ones_sb[:], rec[:],
                                     start=True, stop=True)
                    bcs = small.tile([HD + 1, TCH], f32, tag="bcs",
                                     name=f"bcs{b}_{wc}_{h}")
                    nc.vector.tensor_copy(bcs[:], bc[:])
                    nc.vector.tensor_mul(
                        ch_sb[b][h][:, qoff:qoff + TCH], av[h][:], bcs[:])
                    # partition-shift rows 1-64 -> combined concat
                    nc.sync.dma_start(
                        cc_sb[b][h * HD:(h + 1) * HD, qoff:qoff + TCH],
                        ch_sb[b][h][1:HD + 1, qoff:qoff + TCH])

            def emit_wo(b, wc):
                qoff = wc * TCH
                for j in range(DCH):
                    acc = ps.tile([128, TCH], f32, tag="wo",
                                  name=f"wops{b}_{wc}_{j}")
                    nc.tensor.matmul(
                        acc[:],
                        wo_sb[:, j * 128:(j + 1) * 128],
                        cc_sb[b][:, qoff:qoff + TCH],
                        start=True, stop=True)
                    od = small.tile([128, TCH], f32, tag="od",
                                    name=f"od{b}_{wc}_{j}")
                    nc.vector.tensor_copy(od[:], acc[:])
                    nc.sync.dma_start(
                        out_d[j * 128:(j + 1) * 128,
                              b * N + qoff:b * N + qoff + TCH],
                        od[:])

            av_tiles = {}
            prev = None
            for b in range(B):
                emit_qkv(b)
                for wc in range(N // TCH):
                    qoff = wc * TCH
                    av = [ps.tile([HD + 1, TCH], f32, tag="av",
                                  name=f"av{b}_{wc}_{h}")
                          for h in range(HPC)]
                    av_tiles[(b, wc)] = av
                    exs = {}
                    for kt in range(NKT + 1):
                        if kt < NKT:
                            koff = kt * KT
                            sc = ps.tile([128, 1024], f32, tag="big",
                                         name=f"sc{b}_{wc}_{kt}")
                            for h in range(HPC):
                                nc.tensor.matmul(
                                    sc[:, h * TCH:(h + 1) * TCH],
                                    k_sb[b][h * HD:(h + 1) * HD,
                                            koff:koff + KT],
                                    q_sb[b][h * HD:(h + 1) * HD,
                                            qoff:qoff + TCH],
                                    start=True, stop=True)
                            ex = expp.tile([128, 1024], bf16, tag="e",
                                           name=f"ex{b}_{wc}_{kt}")
                            nc.scalar.activation(
                                ex[:], sc[:],
                                mybir.ActivationFunctionType.Exp,
                                scale=float(SCALE))
                            exs[kt] = ex
                        if kt > 0:
                            ex = exs.pop(kt - 1)
                            for h in range(HPC):
                                g = (kt - 1) * 2 * (HD + 1) + h * (HD + 1)
                                nc.tensor.matmul(
                                    av[h][:],
                                    v_sb[b][:, g:g + HD + 1],
                                    ex[:, h * TCH:(h + 1) * TCH],
                                    start=(kt == 1), stop=(kt == NKT))
                        if kt == 5 and prev is not None:
                            emit_normalize(*prev)
                        if kt == 14 and prev is not None:
                            emit_wo(*prev)
                            prev = None
                    prev = (b, wc)
            emit_normalize(*prev)
            emit_wo(*prev)
    nc.compile()
    return nc


def _get_built():
    global _BUILT
    if _BUILT is None:
        _BUILT = _build()
    return _BUILT


_RUNNER = None


def _get_runner():
    """Build the sharded jitted executable once (jax.jit caches on function
    identity, so run_bass_kernel_spmd would re-trace every call)."""
    global _RUNNER
    if _RUNNER is not None:
        return _RUNNER
    import jax
    import numpy as _np
    from jax.sharding import Mesh, PartitionSpec
    from jax.experimental.shard_map import shard_map
    from concourse import mybir
    from concourse.bass2jax import (
        install_neuronx_cc_hook, _bass_exec_p, partition_id_tensor)

    nc = _get_built()
    install_neuronx_cc_hook()

    partition_name = (nc.partition_id_tensor.name
                      if nc.partition_id_tensor else None)
    in_names, out_names, out_avals, zero_shapes = [], [], [], []
    for alloc in nc.m.functions[0].allocations:
        if not isinstance(alloc, mybir.MemoryLocationSet):
            continue
        name = alloc.memorylocations[0].name
        if alloc.kind == "ExternalInput":
            if name != partition_name:
                in_names.append(name)
        elif alloc.kind == "ExternalOutput":
            np_dt = mybir.dt.np(alloc.dtype)
            out_avals.append(
                jax.core.ShapedArray(tuple(alloc.tensor_shape), np_dt))
            out_names.append(name)
            zero_shapes.append((tuple(alloc.tensor_shape), np_dt))
    n_params = len(in_names)
    all_names = in_names + out_names
    if partition_name is not None:
        all_names = all_names + [partition_name]
    # xT is identical on every core — pass it replicated
    repl = {"xT"}
    import jax.numpy as jnp
    from jax.sharding import NamedSharding

    def _body(*args):
        operands = list(args)
        if partition_name is not None:
            operands.append(partition_id_tensor())
        outs = _bass_exec_p.bind(
            *operands,
            out_avals=tuple(out_avals),
            in_names=tuple(all_names),
            out_names=tuple(out_names),
            lowering_input_output_aliases=(),
            sim_require_finite=True,
            sim_require_nnan=True,
            nc=nc,
        )
        return tuple(outs)

    devices = jax.devices()[:N_CORES]
    mesh = Mesh(_np.asarray(devices), ("core",))
    n_outs = len(out_names)
    in_specs = tuple(
        PartitionSpec() if nm in repl else PartitionSpec("core")
        for nm in in_names) + (PartitionSpec("core"),) * n_outs
    sharded = jax.jit(
        shard_map(_body, mesh=mesh, in_specs=in_specs,
                  out_specs=(PartitionSpec("core"),) * n_outs,
                  check_rep=False),
        donate_argnums=tuple(range(n_params, n_params + n_outs)),
        keep_unused=True,
    )
    shard0 = NamedSharding(mesh, PartitionSpec("core"))
    zeros_jit = jax.jit(
        lambda: tuple(jnp.zeros((N_CORES * s[0], *s[1:]), dt)
                      for s, dt in zero_shapes),
        out_shardings=tuple(shard0 for _ in zero_shapes))
    # sum the 8 partial W_o products on-device (all-reduce), return one copy
    reduce_jit = jax.jit(
        lambda o: jnp.sum(o.reshape(N_CORES, *zero_shapes[0][0]), axis=0),
        out_shardings=NamedSharding(mesh, PartitionSpec()))

    def run(in_maps):
        args = []
        for nm in in_names:
            if nm in repl:
                args.append(in_maps[0][nm])
            else:
                args.append(_np.concatenate(
                    [in_maps[c][nm] for c in range(N_CORES)], axis=0))
        outs = sharded(*args, *zeros_jit())
        return _np.asarray(reduce_jit(outs[0]))

    _RUNNER = run
    return run


def _head_rows(h):
    # feature d of head h sits at column i*H + h of the QKV projection
    # output (einops 'b n (d h)' with head as the inner factor)
    return np.arange(HD) * H + h


def shard_inputs(x, W_q, W_k, W_v, W_o):
    bf = ml_dtypes.bfloat16
    xT = np.ascontiguousarray(
        np.asarray(x, dtype=np.float32).reshape(T, D).T).astype(bf)
    W_q, W_k, W_v, W_o = (np.asarray(w, dtype=np.float32)
                          for w in (W_q, W_k, W_v, W_o))
    in_maps = []
    for c in range(N_CORES):
        rows = np.concatenate([_head_rows(2 * c), _head_rows(2 * c + 1)])
        # concat ('b h n d -> b n (h d)') puts head h's features at
        # columns [h*64, (h+1)*64)
        cols = np.arange(2 * c * HD, (2 * c + 2) * HD)
        in_maps.append({
            "xT": xT,
            "wq": np.ascontiguousarray(W_q[rows, :].T).astype(bf),
            "wk": np.ascontiguousarray(W_k[rows, :].T).astype(bf),
            "wv": np.ascontiguousarray(W_v[rows, :].T).astype(bf),
            "wo": np.ascontiguousarray(W_o[:, cols].T).astype(bf),
        })
    return in_maps


def kernel(x, W_q, W_k, W_v, W_o):
    run = _get_runner()
    total = run(shard_inputs(x, W_q, W_k, W_v, W_o))
    return np.ascontiguousarray(total.T).reshape(B, N, D)


# revision 31
# speedup vs baseline: 1.0888x; 1.0836x over previous
"""Multi-head attention (B=2, N=2048, D=1024, H=16) on 8 trn2 cores.

Sharding: tensor-parallel over heads — each core computes 2 heads' QKV
projections + attention + its partial W_o product; the host sums the 8
partials (the all-reduce of the W_o row-sharding, done at unshard time).

Per-core DRAM layouts (feature-major / transposed):
  xT   [1024 d_in, 4096 tok]          bf16
  wq/wk/wv [1024 d_in, 128 f]         bf16 (f = 2 heads x 64, head-major)
  wo   [130, 1024]                    bf16 (two 65-row blocks per head:
                                       row 0 zero, rows 1-64 = W_o slice)
  out  [1024 d_out, 4096 tok]         fp32 (partial; host sums over cores)

Attention computes scores transposed ([keys, queries] in PSUM) so the
softmax denominator is a matmul contraction: V carries a leading ones
column (lhsT [128 k, 65] = [ones | V_h]), so the attn@V accumulator's
row 0 is the denominator and rows 1-64 the weighted values. The
normalization multiplies rows 0-64 by a rank-1 broadcast of 1/denom
(row 0 becomes 1, absorbed by the zero row in wo).
"""
import sys

sys.path.insert(0, "/opt/trn_rl_repo")

import numpy as np
import ml_dtypes

B, N, D, H = 2, 2048, 1024, 16
HD = D // H          # 64
N_CORES = 8
HPC = H // N_CORES   # heads per core = 2
F = HPC * HD         # per-core features = 128
T = B * N            # tokens = 4096
TCH = 512            # token chunk (QKV phase, q-window)
KT = 128             # key tile
NKT = N // KT        # 16 key tiles per batch
SCALE = 1.0 / np.sqrt(HD)

DCH_H = D // 128
_BUILT = None


def _build():
    import concourse.tile as tile
    from concourse import bacc, mybir

    bf16 = mybir.dt.bfloat16
    f32 = mybir.dt.float32

    nc = bacc.Bacc("TRN2", target_bir_lowering=False, debug=False,
                   num_devices=N_CORES)
    xT_d = nc.dram_tensor("xT", [D, T], bf16, kind="ExternalInput").ap()
    wq_d = nc.dram_tensor("wq", [128, D], bf16, kind="ExternalInput").ap()
    wk_d = nc.dram_tensor("wk", [128, D], bf16, kind="ExternalInput").ap()
    wv_d = nc.dram_tensor("wv", [128, D], bf16, kind="ExternalInput").ap()
    wo_d = nc.dram_tensor("wo", [F, D], bf16, kind="ExternalInput").ap()
    out_d = nc.dram_tensor("out", [D, T], f32, kind="ExternalOutput").ap()

    DCH = D // 128  # 8 contraction chunks for the projections

    with tile.TileContext(nc) as tc:
        with (
            tc.tile_pool(name="wpool", bufs=1) as wpool,
            tc.tile_pool(name="xin", bufs=16) as xin,
            tc.tile_pool(name="qkv", bufs=1) as qkv,
            tc.tile_pool(name="expp", bufs=3) as expp,
            tc.tile_pool(name="small", bufs=4) as small,
            tc.tile_pool(name="ps", bufs=2, space="PSUM") as ps,
        ):
            # --- weights to SBUF (host pre-rearranges wq/wk/wv into the
            # on-chip [128, c*F] layout so the DMA is contiguous) ---
            wq_sb = wpool.tile([128, D], bf16, tag="wq")
            wk_sb = wpool.tile([128, D], bf16, tag="wk")
            wv_sb = wpool.tile([128, D], bf16, tag="wv")
            wo_sb = wpool.tile([F, D], bf16, tag="wo")
            for w_sb, w_d in ((wq_sb, wq_d), (wk_sb, wk_d), (wv_sb, wv_d),
                              (wo_sb, wo_d)):
                nc.gpsimd.dma_start(w_sb[:], w_d[:])
            ones_sb = wpool.tile([1, HD + 1], f32, tag="ones")
            nc.vector.memset(ones_sb[:], 1.0)

            # --- per-batch persistent tensors ---
            q_sb = [qkv.tile([128, N], bf16, tag=f"q{b}", name=f"q{b}")
                    for b in range(B)]
            k_sb = [qkv.tile([128, N], bf16, tag=f"k{b}", name=f"k{b}")
                    for b in range(B)]
            # v: per batch [128 tok, 16 ktiles x 2 heads x (ones|64 vals)]
            VW = NKT * 2 * (HD + 1)  # 2080
            v_sb = [qkv.tile([128, VW], bf16, tag=f"v{b}", name=f"v{b}")
                    for b in range(B)]
            # per-head normalized output (row 0 garbage), then DMA-shifted
            # into the combined concat [128 f, N] for a single K=128 W_o
            ch_sb = [[qkv.tile([HD + 1, N], bf16, tag=f"h{b}{h}",
                               name=f"h{b}{h}")
                      for h in range(HPC)] for b in range(B)]
            cc_sb = [qkv.tile([F, N], bf16, tag=f"c{b}", name=f"c{b}")
                     for b in range(B)]

            for b in range(B):
                nc.vector.memset(
                    v_sb[b].rearrange("p (g c) -> p g c", c=HD + 1)[:, :, 0:1],
                    1.0)

            # --- QKV emission, split into sub-passes so batch 1's
            # projections can be injected into batch 0's (ACT-bound)
            # attention loop. Accumulators rotate through the short-lived
            # "wo" PSUM tag so they never block the scores pipeline. ---
            xbufs = {}

            def emit_xload(b):
                xb = [xin.tile([128, N], bf16, tag="x", name=f"x{b}_{c}")
                      for c in range(DCH)]
                for c in range(DCH):
                    nc.sync.dma_start(
                        xb[c][:], xT_d[c * 128:(c + 1) * 128,
                                       b * N:(b + 1) * N])
                xbufs[b] = xb

            def emit_qk_pass(b, w_sb, dst, wc, nm):
                xb = xbufs[b]
                acc = ps.tile([128, TCH], f32, tag="wo",
                              name=f"qk{b}_{nm}_{wc}")
                for c in range(DCH):
                    nc.tensor.matmul(
                        acc[:], w_sb[:, c * 128:(c + 1) * 128],
                        xb[c][:, wc * TCH:(wc + 1) * TCH],
                        start=(c == 0), stop=(c == DCH - 1))
                nc.vector.tensor_copy(
                    dst[:, wc * TCH:(wc + 1) * TCH], acc[:])

            def emit_v_pass(b, wc):
                xb = xbufs[b]
                for tt4 in range(TCH // 128):
                    tt = wc * 4 + tt4
                    acc = ps.tile([128, TCH], f32, tag="wo",
                                  name=f"vp{b}_{tt}")
                    for c in range(DCH):
                        nc.tensor.matmul(
                            acc[:, 0:128],
                            xb[c][:, tt * 128:(tt + 1) * 128],
                            wv_sb[:, c * 128:(c + 1) * 128],
                            start=(c == 0), stop=(c == DCH - 1))
                    for h in range(HPC):
                        g = tt * 2 * (HD + 1) + h * (HD + 1)
                        nc.vector.tensor_copy(
                            v_sb[b][:, g + 1:g + HD + 1],
                            acc[:, h * HD:(h + 1) * HD])

            def qkv_passes(b):
                for wc in range(N // TCH):
                    yield lambda wc=wc: emit_qk_pass(b, wq_sb, q_sb[b],
                                                     wc, "q")
                    yield lambda wc=wc: emit_qk_pass(b, wk_sb, k_sb[b],
                                                     wc, "k")
                    yield lambda wc=wc: emit_v_pass(b, wc)

            def emit_normalize(b, wc):
                qoff = wc * TCH
                av = av_tiles.pop((b, wc))
                for h in range(HPC):
                    rec = small.tile([1, TCH], f32, tag="rec",
                                     name=f"rec{b}_{wc}_{h}")
                    nc.vector.reciprocal_approx_fast(rec[:], av[h][0:1, :])
                    bc = ps.tile([HD + 1, TCH], f32, tag="wo",
                                 name=f"bc{b}_{wc}_{h}")
                    nc.tensor.matmul(bc[:], ones_sb[:], rec[:],
                                     start=True, stop=True)
                    bcs = small.tile([HD + 1, TCH], f32, tag="bcs",
                                     name=f"bcs{b}_{wc}_{h}")
                    nc.vector.tensor_copy(bcs[:], bc[:])
                    nc.vector.tensor_mul(
                        ch_sb[b][h][:, qoff:qoff + TCH], av[h][:], bcs[:])
                    nc.sync.dma_start(
                        cc_sb[b][h * HD:(h + 1) * HD, qoff:qoff + TCH],
                        ch_sb[b][h][1:HD + 1, qoff:qoff + TCH])

            def emit_wo(b, wc):
                qoff = wc * TCH
                for j in range(DCH):
                    acc = ps.tile([128, TCH], f32, tag="wo",
                                  name=f"wops{b}_{wc}_{j}")
                    nc.tensor.matmul(
                        acc[:],
                        wo_sb[:, j * 128:(j + 1) * 128],
                        cc_sb[b][:, qoff:qoff + TCH],
                        start=True, stop=True)
                    od = small.tile([128, TCH], f32, tag="od",
                                    name=f"od{b}_{wc}_{j}")
                    nc.vector.tensor_copy(od[:], acc[:])
                    nc.sync.dma_start(
                        out_d[j * 128:(j + 1) * 128,
                              b * N + qoff:b * N + qoff + TCH],
                        od[:])

            # --- flat attention pipeline over all (batch, window) pairs
            # with a one-ktile skew between scores/exp and attn@V, so the
            # PE streams ahead of the ACT-bound exp chain ---
            av_tiles = {}
            wins = [(b, wc) for b in range(B) for wc in range(N // TCH)]
            emit_xload(0)
            for p in qkv_passes(0):
                p()
            emit_xload(1)
            inject = {}  # (win_idx, kt) -> list of emit fns
            for i, p in enumerate(qkv_passes(1)):
                w_i, slot = divmod(i, 3)
                inject.setdefault((w_i, 7 + 3 * slot), []).append(p)

            def sc_exp(w_i, kt):
                b, wc = wins[w_i]
                qoff = wc * TCH
                if kt == 0:
                    av_tiles[(b, wc)] = [
                        ps.tile([HD + 1, TCH], f32, tag="av",
                                name=f"av{b}_{wc}_{h}")
                        for h in range(HPC)]
                koff = kt * KT
                sc = ps.tile([128, 1024], f32, tag="big",
                             name=f"sc{b}_{wc}_{kt}")
                for h in range(HPC):
                    nc.tensor.matmul(
                        sc[:, h * TCH:(h + 1) * TCH],
                        k_sb[b][h * HD:(h + 1) * HD, koff:koff + KT],
                        q_sb[b][h * HD:(h + 1) * HD, qoff:qoff + TCH],
                        start=True, stop=True)
                ex = expp.tile([128, 1024], bf16, tag="e",
                               name=f"ex{b}_{wc}_{kt}")
                nc.scalar.activation(
                    ex[:], sc[:], mybir.ActivationFunctionType.Exp,
                    scale=float(SCALE))
                return ex

            def av_mm(w_i, kt, ex):
                b, wc = wins[w_i]
                av = av_tiles[(b, wc)]
                for h in range(HPC):
                    g = kt * 2 * (HD + 1) + h * (HD + 1)
                    nc.tensor.matmul(
                        av[h][:],
                        v_sb[b][:, g:g + HD + 1],
                        ex[:, h * TCH:(h + 1) * TCH],
                        start=(kt == 0), stop=(kt == NKT - 1))

            NW = len(wins)
            steps = NW * NKT
            pend = None  # (w_i, kt, ex) awaiting its attn@V
            for s in range(steps + 1):
                if s < steps:
                    w_i, kt = divmod(s, NKT)
                    ex = sc_exp(w_i, kt)
                else:
                    w_i, kt, ex = NW, 0, None
                if pend is not None:
                    av_mm(*pend)
                pend = (w_i, kt, ex) if ex is not None else None
                if kt == 5 and w_i > 0:
                    emit_normalize(*wins[w_i - 1])
                if kt == 14 and w_i > 0:
                    emit_wo(*wins[w_i - 1])
                for fn in inject.get((w_i, kt), []):
                    fn()
            emit_normalize(*wins[-1])
            emit_wo(*wins[-1])
    nc.compile()
    return nc


def _get_built():
    global _BUILT
    if _BUILT is None:
        _BUILT = _build()
    return _BUILT


_RUNNER = None


def _get_runner():
    """Build the sharded jitted executable once (jax.jit caches on function
    identity, so run_bass_kernel_spmd would re-trace every call)."""
    global _RUNNER
    if _RUNNER is not None:
        return _RUNNER
    import jax
    import numpy as _np
    from jax.sharding import Mesh, PartitionSpec
    from jax.experimental.shard_map import shard_map
    from concourse import mybir
    from concourse.bass2jax import (
        install_neuronx_cc_hook, _bass_exec_p, partition_id_tensor)

    nc = _get_built()
    install_neuronx_cc_hook()

    partition_name = (nc.partition_id_tensor.name
                      if nc.partition_id_tensor else None)
    in_names, out_names, out_avals, zero_shapes = [], [], [], []
    for alloc in nc.m.functions[0].allocations:
        if not isinstance(alloc, mybir.MemoryLocationSet):
            continue
        name = alloc.memorylocations[0].name
        if alloc.kind == "ExternalInput":
            if name != partition_name:
                in_names.append(name)
        elif alloc.kind == "ExternalOutput":
            np_dt = mybir.dt.np(alloc.dtype)
            out_avals.append(
                jax.core.ShapedArray(tuple(alloc.tensor_shape), np_dt))
            out_names.append(name)
            zero_shapes.append((tuple(alloc.tensor_shape), np_dt))
    n_params = len(in_names)
    all_names = in_names + out_names
    if partition_name is not None:
        all_names = all_names + [partition_name]
    # xT is identical on every core — pass it replicated
    repl = {"xT"}
    import jax.numpy as jnp
    from jax.sharding import NamedSharding

    def _body(*args):
        operands = list(args)
        if partition_name is not None:
            operands.append(partition_id_tensor())
        outs = _bass_exec_p.bind(
            *operands,
            out_avals=tuple(out_avals),
            in_names=tuple(all_names),
            out_names=tuple(out_names),
            lowering_input_output_aliases=(),
            sim_require_finite=True,
            sim_require_nnan=True,
            nc=nc,
        )
        return tuple(outs)

    devices = jax.devices()[:N_CORES]
    mesh = Mesh(_np.asarray(devices), ("core",))
    n_outs = len(out_names)
    in_specs = tuple(
        PartitionSpec() if nm in repl else PartitionSpec("core")
        for nm in in_names) + (PartitionSpec("core"),) * n_outs
    sharded = jax.jit(
        shard_map(_body, mesh=mesh, in_specs=in_specs,
                  out_specs=(PartitionSpec("core"),) * n_outs,
                  check_rep=False),
        donate_argnums=tuple(range(n_params, n_params + n_outs)),
        keep_unused=True,
    )
    shard0 = NamedSharding(mesh, PartitionSpec("core"))
    zeros_jit = jax.jit(
        lambda: tuple(jnp.zeros((N_CORES * s[0], *s[1:]), dt)
                      for s, dt in zero_shapes),
        out_shardings=tuple(shard0 for _ in zero_shapes))
    # sum the 8 partial W_o products on-device (all-reduce), return one copy
    reduce_jit = jax.jit(
        lambda o: jnp.sum(o.reshape(N_CORES, *zero_shapes[0][0]), axis=0),
        out_shardings=NamedSharding(mesh, PartitionSpec()))

    def run(in_maps):
        args = []
        for nm in in_names:
            if nm in repl:
                args.append(in_maps[0][nm])
            else:
                args.append(_np.concatenate(
                    [in_maps[c][nm] for c in range(N_CORES)], axis=0))
        outs = sharded(*args, *zeros_jit())
        return _np.asarray(reduce_jit(outs[0]))

    _RUNNER = run
    return run


def _head_rows(h):
    # feature d of head h sits at column i*H + h of the QKV projection
    # output (einops 'b n (d h)' with head as the inner factor)
    return np.arange(HD) * H + h


def shard_inputs(x, W_q, W_k, W_v, W_o):
    bf = ml_dtypes.bfloat16
    xT = np.ascontiguousarray(
        np.asarray(x, dtype=np.float32).reshape(T, D).T).astype(bf)
    W_q, W_k, W_v, W_o = (np.asarray(w, dtype=np.float32)
                          for w in (W_q, W_k, W_v, W_o))
    in_maps = []
    for c in range(N_CORES):
        rows = np.concatenate([_head_rows(2 * c), _head_rows(2 * c + 1)])
        # concat ('b h n d -> b n (h d)') puts head h's features at
        # columns [h*64, (h+1)*64)
        cols = np.arange(2 * c * HD, (2 * c + 2) * HD)
        def chip_layout(w):
            # [1024 d, 128 f] -> [128 p, 8 chunks * 128 f]
            wt = w[rows, :].T.reshape(DCH_H, 128, F)
            return np.ascontiguousarray(
                wt.transpose(1, 0, 2).reshape(128, D)).astype(bf)
        in_maps.append({
            "xT": xT,
            "wq": chip_layout(W_q),
            "wk": chip_layout(W_k),
            "wv": chip_layout(W_v),
            "wo": np.ascontiguousarray(W_o[:, cols].T).astype(bf),
        })
    return in_maps


def kernel(x, W_q, W_k, W_v, W_o):
    run = _get_runner()
    total = run(shard_inputs(x, W_q, W_k, W_v, W_o))
    return np.ascontiguousarray(total.T).reshape(B, N, D)
